# revision 53
# baseline (speedup 1.0000x reference)
"""Multi-head self-attention (B=2, S=2048, E=1024, H=16) on 8 TRN2 cores.

Sharding: batch (2) x head-groups (4) -> 8 cores. Core c handles batch
c//4 and heads [4*(c%4), 4*(c%4)+4). Each core computes QKV projection,
attention, and its partial output projection; the host sums the 4
head-group partials per batch.

Device schedule (per core, bf16 matmuls, fp32 accumulation):
  - x arrives pre-transposed (E, S); q/k are produced feature-major
    (dh on partitions, head pair stacked 64+64) so the two score matmuls
    of a pair run CONCURRENTLY in disjoint PE row groups
    (tile_position (0,0)/(64,0)), halving score PE time.
  - exp of scores is split across ScalarE (exact Exp) and VectorE
    (Schraudolph int16 bit-trick: i16 = round(s*A + B) reinterpreted as
    bf16 approximates exp to ~1.8% rms) so neither engine is the
    bottleneck. A deep SBUF queue of exp'd tiles decouples the
    score/exp "lead" stream from the AV "trail" stream.
  - v is token-major with an interleaved ones column so attn^T.T@[v|1]
    yields attention output and softmax row-sums in one accumulation.
  - normalization: rowsum reciprocals as exp(-ln x) on ScalarE (one
    ACT table set), broadcast through DRAM on the gpsimd DMA queue, and
    the normalize multiply runs on GpSimd to keep DVE free for exp.
    Steady-state blocks read the rowsum row from the SBUF evacuation copy
    (so the PSUM bank frees after the DVE copy alone); the final g-block
    reads it straight from PSUM, broadcasts 1/rowsum with K=1 ones
    matmuls into PSUM, and normalizes on DVE — no DMA on the tail chain.
  - outT is per-(pair, 512-query block) so each outproj(g) depends only
    on its own block's normalize; outproj(2)/(3) are emitted at the
    (g3,p0)/(g3,p1) block ends so a single out-projection sits on the
    kernel tail, with its pair-0 matmuls hoisted under the final chain.
  - phase1 copy-out alternates DVE/ACT j-major so the first score unit
    isn't gated behind 8 serial DVE casts; the tail skips the stock
    tile-context sem clear + second barrier (preamble re-inits sems).
"""

from contextlib import ExitStack

import numpy as np
import ml_dtypes

import concourse.bass as bass
import concourse.tile as tile
from concourse import mybir
from concourse.vector_clock import ScopedClock
from concourse.bass_utils import run_bass_kernel_spmd

B, S, E = 2, 2048, 1024
H, DH = 16, 64
NCORES = 8
HL = 4              # heads per core
GF = HL * DH        # 256: local head feature dim
VW = DH + 1         # v block width incl. ones column
BF16 = mybir.dt.bfloat16
F32 = mybir.dt.float32
I16 = mybir.dt.int16
bf16 = ml_dtypes.bfloat16

P = 128
EK = E // P         # 8 contraction chunks
ST = S // P         # 16 key tiles
SQ = S // 512       # 4 query chunks

SCALE = float(DH) ** -0.5
A_SCH = 128 * np.log2(np.e) * SCALE   # schraudolph mult (incl. score scale)
B_SCH = 127 * 128 - 7.5               # schraudolph bias (bf16 bit domain)
DVE_IK = frozenset((1, 3, 5, 7, 9, 11, 14))  # 7/16 of exps on DVE


def _dedupe_ldweights(nc):
    """Drop PE LDWEIGHTS whose weights are already resident in the array.

    bass emits one InstLdweights per matmul, and walrus's own dedup pass is
    hardcoded off (and rejects explicit InstLdweights anyway). Only PE
    LDWEIGHTS instructions modify the weight array, so a load identical to
    the immediately-preceding PE load — with only matmuls/noops between —
    re-loads resident data. This halves weight-port traffic in the QKV
    projection (4 matmuls per stationary) and out-projection (2 per).
    Loads carrying sem waits are kept: Tile attaches a stationary-rewrite
    wait to the reading LDWEIGHTS, so a waitless duplicate is safe. Sem
    updates on a dropped load move onto a same-engine NoOp.
    """
    PE = mybir.EngineType.PE

    def key(inst):
        pap = inst.ins[0]
        return (
            pap.memref,
            pap.offset,
            str(pap.ap),
            str(pap.dtype),
            inst.tile_position,
            inst.perf_mode,
            inst.is_transpose,
        )

    for f in nc.m.functions:
        for bb in f.blocks:
            out = []
            last = None
            for inst in bb.instructions:
                if getattr(inst, "engine", None) != PE:
                    out.append(inst)
                    continue
                tn = type(inst).__name__
                if tn == "InstLdweights":
                    si = getattr(inst, "sync_info", None)
                    waits = list(si.on_wait) if si and si.on_wait else []
                    ups = list(si.on_update) if si and si.on_update else []
                    if (
                        last is not None
                        and key(inst) == last
                        and not waits
                        and not inst.ins[0].memref.startswith("warm")
                    ):
                        if ups:
                            out.append(
                                mybir.InstNoOp(
                                    name=f"I-{nc.next_id()}",
                                    engine=PE,
                                    sync_info=mybir.SyncInfo(
                                        on_wait=[], on_update=ups
                                    ),
                                    bass_nofuse=True,
                                )
                            )
                        continue
                    last = key(inst)
                    out.append(inst)
                elif tn in ("InstMatmult", "InstNoOp"):
                    out.append(inst)
                else:
                    last = None
                    out.append(inst)
            bb.instructions[:] = out


def _split_excess_waits(nc):
    """Rewrite TPB instructions carrying >1 sem wait.

    This ISA build has a single (wait, update) event slot per 64B TPB
    instruction, but Tile emits instructions with several waits. Excess
    waits move onto same-engine NoOps inserted immediately before the
    instruction — the engine executes its stream in order, so waiting on
    preceding NoOps is equivalent. DMA instructions are exempt (their
    waits live in DGE descriptors, which support several).
    """
    for f in nc.m.functions:
        for bb in f.blocks:
            out = []
            for inst in bb.instructions:
                si = getattr(inst, "sync_info", None)
                waits = list(si.on_wait) if si and si.on_wait else []
                if len(waits) > 1:
                    ups = list(si.on_update) if si.on_update else []
                    assert len(ups) <= 1, f"{inst.name}: multi-update unsupported"
                    for w in waits[:-1]:
                        out.append(
                            mybir.InstNoOp(
                                name=f"I-{nc.next_id()}",
                                engine=inst.engine,
                                sync_info=mybir.SyncInfo(on_wait=[w], on_update=[]),
                                bass_nofuse=True,
                            )
                        )
                    inst.sync_info = mybir.SyncInfo(on_wait=[waits[-1]], on_update=ups)
                out.append(inst)
            bb.instructions[:] = out


class SafeTileContext(tile.TileContext):
    """TileContext whose tail drain splits sem waits across chained SP nops.

    This walrus build rejects >1 sync-wait command on a CTRL instruction;
    the stock tail drain can carry several and fails codegen ("Too many
    sync wait commands"). Semantics are unchanged: SP serially waits on
    every clock sem via nops, then drains and barriers as usual.
    """

    MAX_WAITS_PER_INST = 1

    def _drain_and_barrier(self, tick_clock, wait_clock):
        nc = self.nc
        probe = mybir.InstNoOp(
            name=nc.get_next_instruction_name(), engine=mybir.EngineType.SP
        )
        wait_clock.add_sem_waits(probe, ScopedClock({None: tick_clock.global_clock}))
        waits = list(probe.sync_info.on_wait) if probe.sync_info else []
        k = self.MAX_WAITS_PER_INST
        for i in range(0, len(waits), k):
            nop = nc.sync.nop(nofuse=True, hint="tail_wait")
            nop.ins.sync_info = mybir.SyncInfo(
                on_wait=list(waits[i : i + k]), on_update=[]
            )
        nc.sync.drain()
        nc.all_engine_barrier()
        popped = nc._tile_sem_poison_stack.pop()
        assert popped is self._sem_poison
        # The stock tail also clear_and_free_semaphores + a second barrier
        # (~6us). Skipped: this is the outermost tile context, nothing runs
        # after it in the NEFF, and the preamble re-initializes every sem it
        # uses on the next execution.
        self.sems.allocated()


def _emit(ctx, tc, xt, wqk, wv, wo, y):
    nc = tc.nc
    rc_dram = nc.dram_tensor("rc_dram", [HL, S], F32)
    consts = ctx.enter_context(tc.tile_pool(name="consts", bufs=1))
    at_pool = ctx.enter_context(tc.tile_pool(name="at", bufs=44))
    scratch = ctx.enter_context(tc.tile_pool(name="scratch", bufs=8))
    ps_s_pool = ctx.enter_context(tc.tile_pool(name="ps_s", bufs=3, space="PSUM"))
    ps_acc_pool = ctx.enter_context(tc.tile_pool(name="ps_acc", bufs=2, space="PSUM"))

    # ---- PE warm-up while input DMAs land (HAM un-throttle needs ~4us
    # of sustained matmul activity).
    warm = consts.tile([P, 64], BF16, name="warm")
    nc.vector.memset(warm, 0.0)
    # ~64 cold matmuls ≈ 3.4us: bridges exactly from engine start (~7.5us)
    # to the first input chunks landing (~11us) and trips the HAM busy
    # window so the projection phase starts at full clock.
    ps_w = ps_acc_pool.tile([64, 64], F32, name="psw", tag="acc")
    for _ in range(128):
        nc.tensor.matmul(ps_w, warm, warm[:, 0:64], start=True, stop=True)

    # ---- stage inputs in SBUF; xt/wqk chunks alternate so the e-outer
    # projection loops start as soon as the first chunks land.
    xt_sb = [consts.tile([P, S], BF16, name=f"xt{e}") for e in range(EK)]
    wqk_sb = [consts.tile([P, 2 * GF], BF16, name=f"wqk{e}") for e in range(EK)]
    wv_sb = [consts.tile([P, GF], BF16, name=f"wv{e}") for e in range(EK)]
    for e in range(EK):
        nc.sync.dma_start(out=xt_sb[e], in_=xt[P * e : P * (e + 1), :])
        nc.sync.dma_start(out=wqk_sb[e], in_=wqk[P * e : P * (e + 1), :])
    for e in range(EK):
        nc.sync.dma_start(out=wv_sb[e], in_=wv[P * e : P * (e + 1), :])
    wo_sb = []
    for d in range(2):
        t = consts.tile([P, E], BF16, name=f"wo{d}")
        nc.sync.dma_start(out=t, in_=wo[P * d : P * (d + 1), :])
        wo_sb.append(t)

    # feature-major q/k: tile [128 = pair dh-stacked, S]; m: 0,1 = q pair
    # 0,1; 2,3 = k pair 0,1 (wqk cols: q feats 0:256 head-major, k 256:512)
    qkT_sb = []
    for m in range(4):
        t = consts.tile([P, S], BF16, name=f"qk{m}")
        qkT_sb.append(t)
    v_sb = []
    for it in range(ST):
        vt = consts.tile([P, HL * VW], BF16, name=f"v{it}")
        nc.vector.memset(vt, 1.0)
        v_sb.append(vt)

    def emit_qk_group(m, js, hook=None, pool=None):
        # e-outer: all j accumulators advance chunk-by-chunk so matmuls
        # start as soon as each xt/wqk chunk's DMA lands.
        pss = {
            j: (pool or ps_acc_pool).tile([P, 512], F32, name="psqk", tag="acc")
            for j in js
        }
        for e in range(EK):
            for j in js:
                nc.tensor.matmul(
                    pss[j],
                    wqk_sb[e][:, P * m : P * (m + 1)],
                    xt_sb[e][:, 512 * j : 512 * (j + 1)],
                    start=(e == 0),
                    stop=(e == EK - 1),
                )
            if hook is not None and e % 2 == 1:
                hook()
        # copy-out split across DVE/ACT so both accumulator banks free in
        # one copy-time — the next group's psqk allocations wait on them.
        for i, j in enumerate(js):
            dst = qkT_sb[m][:, 512 * j : 512 * (j + 1)]
            if i % 2 == 0:
                nc.vector.tensor_copy(dst, pss[j])
            else:
                nc.scalar.copy(dst, pss[j])

    def emit_qk_phase1(ms):
        # both pair-0 projections (k and q) run e-outer-global across all
        # 8 PSUM banks: DMA-paced at the front, no idle seams between
        # m-groups, and the lead stream can start right after. The 8
        # accumulators live in 2 double-bank ps_s tiles + 4 ps_acc tiles.
        wide = [ps_s_pool.tile([P, 1024], F32, name="pss", tag="s") for _ in range(3)]
        narrow = [
            ps_acc_pool.tile([P, 512], F32, name="psqk", tag="acc") for _ in range(2)
        ]
        pss = {
            (ms[0], 0): wide[0][:, 0:512],
            (ms[0], 1): wide[0][:, 512:1024],
            (ms[0], 2): wide[1][:, 0:512],
            (ms[0], 3): wide[1][:, 512:1024],
            (ms[1], 0): wide[2][:, 0:512],
            (ms[1], 1): wide[2][:, 512:1024],
            (ms[1], 2): narrow[0],
            (ms[1], 3): narrow[1],
        }
        for e in range(EK):
            for m in ms:
                for j in range(SQ):
                    nc.tensor.matmul(
                        pss[(m, j)],
                        wqk_sb[e][:, P * m : P * (m + 1)],
                        xt_sb[e][:, 512 * j : 512 * (j + 1)],
                        start=(e == 0),
                        stop=(e == EK - 1),
                    )
        # copy-out j-major (lead unit 0 needs both m's j=0 slices first) and
        # alternating DVE/ACT — 8 serial DVE casts would gate the first
        # score matmuls by ~2us.
        for i, (j, m) in enumerate((j, m) for j in range(SQ) for m in ms):
            dst = qkT_sb[m][:, 512 * j : 512 * (j + 1)]
            if i % 2 == 0:
                nc.vector.tensor_copy(dst, pss[(m, j)])
            else:
                nc.scalar.copy(dst, pss[(m, j)])

    def emit_v_group(it):
        ps = ps_acc_pool.tile([P, GF], F32, name="psv", tag="acc")
        for e in range(EK):
            nc.tensor.matmul(
                ps,
                xt_sb[e][:, P * it : P * (it + 1)],
                wv_sb[e],
                start=(e == 0),
                stop=(e == EK - 1),
            )
        dst = v_sb[it].rearrange("p (h c) -> p h c", c=VW)[:, :, 0:DH]
        src = ps.rearrange("p (h c) -> p h c", c=DH)
        nc.vector.tensor_copy(dst, src)

    # ---- lead stream: score pair matmuls (concurrent row groups) + exp
    def emit_lead(u):
        g, p, ik = u >> 5, (u >> 4) & 1, u & 15
        qt = qkT_sb[p]
        kt = qkT_sb[2 + p]
        s = ps_s_pool.tile([P, 1024], F32, name="pss", tag="s")
        for h01 in range(2):
            po = 64 * h01
            nc.tensor.matmul(
                s[:, 512 * h01 : 512 * (h01 + 1)],
                kt[po : po + 64, P * ik : P * (ik + 1)],
                qt[po : po + 64, 512 * g : 512 * (g + 1)],
                start=True,
                stop=True,
                tile_position=(po, 0),
            )
        at = at_pool.tile([P, 1024], BF16, name="at", tag="at")
        if u >= 120:
            # final units sit on the drain critical path: halve the exp
            # latency by splitting the tile across both engines.
            nc.scalar.activation(
                at[:, 0:512], s[:, 0:512],
                mybir.ActivationFunctionType.Exp, scale=SCALE,
            )
            nc.vector.tensor_scalar(
                out=at[:, 512:1024].bitcast(I16),
                in0=s[:, 512:1024],
                scalar1=A_SCH,
                scalar2=B_SCH,
                op0=mybir.AluOpType.mult,
                op1=mybir.AluOpType.add,
            )
        elif ik in DVE_IK:
            nc.vector.tensor_scalar(
                out=at[:, :].bitcast(I16),
                in0=s,
                scalar1=A_SCH,
                scalar2=B_SCH,
                op0=mybir.AluOpType.mult,
                op1=mybir.AluOpType.add,
            )
        else:
            nc.scalar.activation(at, s, mybir.ActivationFunctionType.Exp, scale=SCALE)
        return at

    # ---- trail stream: AV accumulation, normalization, out projection
    # outT is per-(pair, query-block) so outproj(g) depends only on its own
    # block's normalize (shared tiles made outproj(2) wait on g3's writes).
    outT_sb = [
        [consts.tile([P, 512], BF16, name=f"ot{d}g{g}") for g in range(SQ)]
        for d in range(2)
    ]
    # ones row for the K=1 rcp broadcast matmuls on the kernel tail.
    ones_bc = consts.tile([1, 64], BF16, name="ones_bc")
    nc.vector.memset(ones_bc, 1.0)
    rb_pool = ctx.enter_context(tc.tile_pool(name="rb", bufs=6))
    y_pool = ctx.enter_context(tc.tile_pool(name="ystage", bufs=4))

    trail_accs = {}   # (g, p) -> [acc_even, acc_odd]
    at_q = []         # queue of at tiles, lead-filled, trail-consumed

    def emit_trail(t):
        g, p, ik = t >> 5, (t >> 4) & 1, t & 15
        if ik == 0:
            trail_accs[(g, p)] = [
                ps_acc_pool.tile([VW, 512], F32, name=f"pso{h01}", tag="acc")
                for h01 in range(2)
            ]
        accs = trail_accs[(g, p)]
        at = at_q.pop(0)
        for h01 in range(2):
            h = 2 * p + h01
            nc.tensor.matmul(
                accs[h01],
                v_sb[ik][:, VW * h : VW * (h + 1)],
                at[:, 512 * h01 : 512 * (h01 + 1)],
                start=(ik == 0),
                stop=(ik == ST - 1),
            )

    def emit_block_end(g, p):
        # evacuate the two AV accumulators (frees PSUM), extract rowsums,
        # reciprocal + broadcast + normalize — all per pair so only the
        # final pair's chain sits on the kernel tail.
        fast = g == SQ - 1
        accs = trail_accs.pop((g, p))
        rln = scratch.tile([33, 512], F32, name="rln", tag="rsp", bufs=2)
        outUs = [None, None]
        if fast:
            # tail blocks: broadcast rcp across partitions with K=1
            # ones-matmuls (PE is idle here) instead of the DRAM roundtrip,
            # and normalize on DVE -- keeps DMA/GpSimd off the tail chain.
            # The per-head chain (copy, Ln, Exp, bc-matmul) is interleaved
            # so ACT runs Ln0,Exp0,Ln1,Exp1 and h0's broadcast lands ~1.4us
            # earlier than an all-Lns-then-all-Exps order. The very last
            # block gives each head its own bc tile (h1's from the acc
            # pool, which its accumulators vacate mid-chain) so the h0
            # normalize multiply doesn't wait on h1's broadcast matmul via
            # a shared-tile dependency.
            bcs = []
            bc_sh = None
            for h01 in range(2):
                outU = scratch.tile([VW, 512], F32, name="ou", tag="ou", bufs=4)
                nc.vector.tensor_copy(outU, accs[h01])
                outUs[h01] = outU
                rs_src = accs[h01] if p == 1 else outU
                nc.scalar.activation(
                    rln[32 * h01 : 32 * h01 + 1, :],
                    rs_src[DH : DH + 1, :],
                    mybir.ActivationFunctionType.Ln,
                )
                rcp0 = scratch.tile(
                    [1, 512], BF16, name=f"rcp{h01}", tag="rcpbc", bufs=2
                )
                nc.scalar.activation(
                    rcp0,
                    rln[32 * h01 : 32 * h01 + 1, :],
                    mybir.ActivationFunctionType.Exp,
                    scale=-1.0,
                )
                if p == 1:
                    # both bc tiles from the acc pool: this block's own
                    # accumulators die at the evac/Ln reads just before the
                    # bc matmuls need banks, and it leaves all three wide
                    # ps_s slots for pre-emitted outproj d0 accumulation.
                    bc = ps_acc_pool.tile(
                        [64, 512], F32, name=f"bc{h01}", tag="acc"
                    )
                    nc.tensor.matmul(bc, ones_bc, rcp0, start=True, stop=True)
                    bcs.append(bc[0:64, :])
                else:
                    if bc_sh is None:
                        bc_sh = ps_s_pool.tile([P, 512], F32, name="bc", tag="s")
                    nc.tensor.matmul(
                        bc_sh[64 * h01 : 64 * (h01 + 1), :],
                        ones_bc,
                        rcp0,
                        start=True,
                        stop=True,
                        tile_position=(0, 64 * h01),
                    )
                    bcs.append(bc_sh[64 * h01 : 64 * (h01 + 1), :])
            for h01 in range(2):
                po = 64 * h01
                nc.vector.tensor_tensor(
                    out=outT_sb[p][g][po : po + 64, :],
                    in0=outUs[h01][0:DH, :],
                    in1=bcs[h01],
                    op=mybir.AluOpType.mult,
                )
            return
        # h1 first: its acc bank frees ~600ns earlier, which is the bank the
        # NEXT block's first AV matmul grabs (the pool hands out LRU-free).
        for h01 in (1, 0):
            outU = scratch.tile([VW, 512], F32, name="ou", tag="ou", bufs=4)
            nc.vector.tensor_copy(outU, accs[h01])
            # 1/x as exp(-ln x): Ln and Exp share one ACT table set, so this
            # avoids both the slow DVE iterative divide and a table swap.
            # The rowsum row is read from the outU copy so the PSUM bank
            # frees after the DVE copy alone -- an ACT reader would queue
            # behind a 1.1us exp and stall the next block's AV.
            nc.scalar.activation(
                rln[32 * h01 : 32 * h01 + 1, :],
                outU[DH : DH + 1, :],
                mybir.ActivationFunctionType.Ln,
            )
            outUs[h01] = outU
        rcp = scratch.tile([33, 512], F32, name="rcp", tag="rsp", bufs=2)
        nc.scalar.activation(rcp, rln, mybir.ActivationFunctionType.Exp, scale=-1.0)
        for h01 in range(2):
            h = 2 * p + h01
            nc.sync.dma_start(
                out=rc_dram[h : h + 1, 512 * g : 512 * (g + 1)],
                in_=rcp[32 * h01 : 32 * h01 + 1, :],
            )
        rbs = {}
        for h01 in range(2):
            h = 2 * p + h01
            rb = rb_pool.tile([64, 512], F32, name="rb", tag="rb")
            nc.gpsimd.dma_start(
                out=rb,
                in_=rc_dram[h : h + 1, 512 * g : 512 * (g + 1)].partition_broadcast(64),
            )
            rbs[h] = rb
        for h01 in range(2):
            h = 2 * p + h01
            po = 64 * h01
            nc.gpsimd.tensor_tensor(
                out=outT_sb[p][g][po : po + 64, :],
                in0=outUs[h01][0:DH, :],
                in1=rbs[h],
                op=mybir.AluOpType.mult,
            )

    def emit_outproj_d0(g, it2s):
        # first-half (pair 0) accumulation only: these matmuls depend just
        # on outT[0][g], which is ready one block earlier. Emitted BEFORE
        # the final block_end so they sit AHEAD of the bc matmuls in the
        # in-order PE queue and fill the reciprocal chain's ~2.5us.
        tiles = {}
        for it2 in it2s:
            ps_yw = ps_s_pool.tile([P, 1024], F32, name="psy", tag="s")
            for u in range(2):
                nc.tensor.matmul(
                    ps_yw[:, 512 * u : 512 * (u + 1)],
                    outT_sb[0][g][:, P * it2 : P * (it2 + 1)],
                    wo_sb[0][:, 512 * u : 512 * (u + 1)],
                    start=True,
                    stop=False,
                )
            tiles[it2] = ps_yw
        return tiles

    def emit_outproj(g, d0_tiles=None):
        # partial y chunk for tokens [512g, 512(g+1)) = outT.T @ wo.
        # d-outer: one stationary outT chunk serves both wo halves.
        d0_tiles = d0_tiles or {}
        for it2 in range(4):
            it = 4 * g + it2
            y_sb = y_pool.tile([P, E], BF16, name="ysb", tag="y")
            ps_yw = d0_tiles.get(it2)
            if ps_yw is None:
                ps_yw = ps_s_pool.tile([P, 1024], F32, name="psy", tag="s")
                ds = (0, 1)
            else:
                ds = (1,)
            for d in ds:
                for u in range(2):
                    nc.tensor.matmul(
                        ps_yw[:, 512 * u : 512 * (u + 1)],
                        outT_sb[d][g][:, P * it2 : P * (it2 + 1)],
                        wo_sb[d][:, 512 * u : 512 * (u + 1)],
                        start=(d == 0),
                        stop=(d == 1),
                    )
            tail = g == SQ - 1
            for u in range(2):
                if u == 0:
                    nc.vector.tensor_copy(
                        y_sb[:, 512 * u : 512 * (u + 1)],
                        ps_yw[:, 512 * u : 512 * (u + 1)],
                    )
                else:
                    nc.scalar.copy(
                        y_sb[:, 512 * u : 512 * (u + 1)],
                        ps_yw[:, 512 * u : 512 * (u + 1)],
                    )
                if tail:
                    # drain each half as soon as it lands so the final DMA
                    # isn't serialized behind both evacuation copies.
                    nc.sync.dma_start(
                        out=y[P * it : P * (it + 1), 512 * u : 512 * (u + 1)],
                        in_=y_sb[:, 512 * u : 512 * (u + 1)],
                    )
            if not tail:
                nc.sync.dma_start(out=y[P * it : P * (it + 1), :], in_=y_sb)

    # ---- emission: projections with lead units interleaved, then the
    # steady lead/trail interleave (trail lags lead by the at queue).
    emit_qk_phase1([2, 0])           # kT + qT pair 0, all query blocks

    proj_rest = (
        [("qk", 3, [0, 1]), ("qk", 3, [2, 3]), ("qk", 1, [0, 1])]
        + [("v", it) for it in range(8)]
        + [("qk", 1, [2, 3])]
        + [("v", it) for it in range(8, ST)]
    )
    lead_state = [0]

    def lead_hook():
        if lead_state[0] < 38:
            at_q.append(emit_lead(lead_state[0]))
            lead_state[0] += 1

    for grp in proj_rest:
        if grp[0] == "qk":
            emit_qk_group(grp[1], grp[2], hook=lead_hook)
        else:
            emit_v_group(grp[1])
            lead_hook()
    lead_u = lead_state[0]

    # steady state: 3 lead + 4 trail per round, with norm/outproj at
    # block boundaries.
    trail_t = 0
    def trail_step():
        """Returns True if a block boundary was just emitted (the caller
        should switch to lead emission so the PE queue has score work to
        chew on while the next block's accumulator banks free up)."""
        nonlocal trail_t
        if trail_t < 128 and len(at_q) > 0:
            emit_trail(trail_t)
            trail_t += 1
            if trail_t % 16 == 0:
                g, p = (trail_t - 1) >> 5, ((trail_t - 1) >> 4) & 1
                last = g == SQ - 1 and p == 1
                # outproj(g) needs both pairs of g. Early blocks defer one
                # block so outproj PSUM doesn't fight the score pool. The
                # last two blocks end with bc matmuls that wait on the
                # reciprocal ACT chain, and the in-order PE queue parks
                # everything emitted after them — so work whose inputs are
                # already ready (all of outproj(2); outproj(3)'s pair-0
                # accumulation) is emitted BEFORE the block_end.
                if g == SQ - 1 and p == 0:
                    emit_outproj(g - 1)
                d0_tiles = emit_outproj_d0(g, (0, 1, 2)) if last else None
                emit_block_end(g, p)
                if p == 1 and g in (1, 2):
                    emit_outproj(g - 1)
                elif last:
                    emit_outproj(g, d0_tiles)
                return True
        return False

    # round shape [L L L T T T T]: three score-pair tiles rotate through
    # the 3-deep ps_s pool, so consecutive pairs issue back-to-back and
    # their kt weight loads pull ahead into the half-row score matmuls
    # instead of stalling behind full-row AV matmuls.
    while lead_u < 128 or trail_t < 128:
        # 4 leads per round once the lead stream nears its end, so the last
        # exps land well before the trail drains and never gate the tail.
        for _ in range(3 if lead_u < 98 else 4):
            if lead_u < 128:
                at_q.append(emit_lead(lead_u))
                lead_u += 1
        for _ in range(4):
            if trail_step():
                break


def build_nc(split_waits=True):
    nc = bass.Bass(trn_type="TRN2")
    xt = nc.dram_tensor("xt", [E, S], BF16, kind="ExternalInput")
    wqk = nc.dram_tensor("wqk", [E, 2 * GF], BF16, kind="ExternalInput")
    wv = nc.dram_tensor("wv", [E, GF], BF16, kind="ExternalInput")
    wo = nc.dram_tensor("wo", [GF, E], BF16, kind="ExternalInput")
    y = nc.dram_tensor("y", [S, E], BF16, kind="ExternalOutput")
    with SafeTileContext(nc) as tc:
        with ExitStack() as ctx:
            _emit(ctx, tc, xt, wqk, wv, wo, y)
    # NOTE: an LDWEIGHTS-dedup post-pass (drop loads identical to the
    # preceding PE load) was tried here and crashes the device
    # (NRT_EXEC_UNIT_UNRECOVERABLE) — matmuls appear to be paired with
    # their own load at lowering (weight-slot assignment). Do not retry.
    if split_waits:
        _split_excess_waits(nc)
    return nc


_NC_CACHE = None


def _get_nc():
    global _NC_CACHE
    if _NC_CACHE is None:
        _NC_CACHE = build_nc()
    return _NC_CACHE


def make_in_maps(x, w_qkv, w_out):
    in_maps = []
    for c in range(NCORES):
        b, g = divmod(c, 4)
        q = w_qkv[GF * g : GF * (g + 1)]
        k = w_qkv[1024 + GF * g : 1024 + GF * (g + 1)]
        v = w_qkv[2048 + GF * g : 2048 + GF * (g + 1)]
        in_maps.append(
            {
                "xt": np.ascontiguousarray(np.asarray(x)[b].T).astype(bf16),
                "wqk": np.ascontiguousarray(
                    np.concatenate([q, k], axis=0).T
                ).astype(bf16),
                "wv": np.ascontiguousarray(np.asarray(v).T).astype(bf16),
                "wo": np.ascontiguousarray(
                    np.asarray(w_out)[:, GF * g : GF * (g + 1)].T
                ).astype(bf16),
            }
        )
    return in_maps


def gather_output(results):
    y = np.zeros((B, S, E), np.float32)
    for c in range(NCORES):
        y[c // 4] += results[c]["y"].astype(np.float32)
    return y


def kernel(x, w_qkv, w_out, **run_kwargs):
    nc = _get_nc()
    in_maps = make_in_maps(np.asarray(x), np.asarray(w_qkv), np.asarray(w_out))
    res = run_bass_kernel_spmd(nc, in_maps, core_ids=list(range(NCORES)), **run_kwargs)
    out = gather_output(res.results)
    if run_kwargs:
        kernel.last_results = res
    return out

